# revision 5
# baseline (speedup 1.0000x reference)
"""Trainium2 Bass kernel for nn_ConditionalSplineSQ2D.

Math:
  out[b] = sum_{g,h,c} coeffs[g,h,c] * p[b,g,h,ii_c] * p[b,g,h,jj_c]
         = sum_{cells} p_cell^T S_cell p_cell            (S_cell symmetric 8x8)
         = sum_{cells} sum_k lam[cell,k] * (V[cell]^T p_cell)_k^2

Host precomputes the eigendecomposition of the 961 8x8 matrices; the device
kernel per 16-cell group does:
  mm1 (PE):    T = Wblk^T @ P          (block-diag stationary, fp16)
  sq  (ACT/DVE): Q = T*T               (PSUM -> SBUF, fp16)
  mm2 (PE):    acc[0,:] += lam_g^T @ Q (accumulating PSUM row)

Sharding: pure data parallel over batch (512 per core x 8 cores).
"""

import numpy as np

B, G, P = 4096, 31, 8
NCORES = 8
NB = B // NCORES  # 512 batches per core
CELLS = G * G  # 961
GROUP_CELLS = 16
NGROUPS = -(-CELLS // GROUP_CELLS)  # 61
CELLS_PAD = NGROUPS * GROUP_CELLS  # 976
PARTS = 128
CH = 8  # groups per DMA chunk

_nc_cache = {}


def _build_nc():
    import concourse.mybir as mybir
    import concourse.tile as tile
    from concourse import bacc

    nc = bacc.Bacc()
    pt_d = nc.dram_tensor(
        "pt", [PARTS, NGROUPS * NB], mybir.dt.float16, kind="ExternalInput"
    )
    w_d = nc.dram_tensor(
        "wblk", [PARTS, NGROUPS * PARTS], mybir.dt.float16, kind="ExternalInput"
    )
    lam_d = nc.dram_tensor(
        "lamt", [PARTS, NGROUPS], mybir.dt.float16, kind="ExternalInput"
    )
    out_d = nc.dram_tensor("out", [1, NB], mybir.dt.float32, kind="ExternalOutput")

    nchunks = -(-NGROUPS // CH)

    with tile.TileContext(nc) as tc:
        with (
            tc.tile_pool(name="const", bufs=1) as cpool,
            tc.tile_pool(name="ptp", bufs=3) as ppool,
            tc.tile_pool(name="qp", bufs=4) as qpool,
            tc.tile_pool(name="psp", bufs=3, space="PSUM") as pspool,
            tc.tile_pool(name="accp", bufs=1, space="PSUM") as apool,
        ):
            w_sb = cpool.tile([PARTS, NGROUPS * PARTS], mybir.dt.float16)
            lam_sb = cpool.tile([PARTS, NGROUPS], mybir.dt.float16)
            nc.sync.dma_start(out=lam_sb[:, :], in_=lam_d[:, :])
            acc = apool.tile([PARTS, NB], mybir.dt.float32)

            q_tiles = {}
            for ci in range(nchunks):
                g0 = ci * CH
                ch = min(CH, NGROUPS - g0)
                nc.sync.dma_start(
                    out=w_sb[:, g0 * PARTS : (g0 + ch) * PARTS],
                    in_=w_d[:, g0 * PARTS : (g0 + ch) * PARTS],
                )
                ptile = ppool.tile([PARTS, CH * NB], mybir.dt.float16, tag="ptile")
                nc.sync.dma_start(
                    out=ptile[:, : ch * NB],
                    in_=pt_d[:, g0 * NB : (g0 + ch) * NB],
                )
                for gg in range(ch):
                    g = g0 + gg
                    psT = pspool.tile([PARTS, NB], mybir.dt.float32)
                    nc.tensor.matmul(
                        psT[:, :],
                        w_sb[:, g * PARTS : (g + 1) * PARTS],
                        ptile[:, gg * NB : (gg + 1) * NB],
                        start=True,
                        stop=True,
                    )
                    q = qpool.tile([PARTS, NB], mybir.dt.float16, tag="q")
                    if g % 3 != 2:
                        # ScalarE: square directly out of PSUM
                        nc.scalar.square(q[:, :], psT[:, :])
                    else:
                        # VectorE: PSUM reads are limited to one operand, so
                        # copy-cast to SBUF fp16 first, then square there
                        qc = qpool.tile([PARTS, NB], mybir.dt.float16, tag="qc")
                        nc.vector.tensor_copy(qc[:, :], psT[:, :])
                        nc.vector.tensor_mul(q[:, :], qc[:, :], qc[:, :])
                    q_tiles[g] = q
                    # software-pipeline: weight-reduce the PREVIOUS group's
                    # squares so PE never waits on the square of group g
                    if g >= 1:
                        gp = g - 1
                        nc.tensor.matmul(
                            acc[0:1, :],
                            lam_sb[:, gp : gp + 1],
                            q_tiles.pop(gp)[:, :],
                            start=(gp == 0),
                            stop=False,
                        )
            gp = NGROUPS - 1
            nc.tensor.matmul(
                acc[0:1, :],
                lam_sb[:, gp : gp + 1],
                q_tiles.pop(gp)[:, :],
                start=False,
                stop=True,
            )
            out_sb = cpool.tile([1, NB], mybir.dt.float32)
            nc.vector.tensor_copy(out_sb[:, :], acc[0:1, :])
            nc.sync.dma_start(out=out_d[:, :], in_=out_sb[:, :])
    if not nc.is_finalized():
        nc.finalize()
    return nc


def _get_nc():
    if "nc" not in _nc_cache:
        _nc_cache["nc"] = _build_nc()
    return _nc_cache["nc"]


def _host_prep_weights(integral_coeffs):
    """coeffs [G,G,C] -> (wblk [128, NGROUPS*128] fp16, lamt [128, NGROUPS] fp16)."""
    ii, jj = np.triu_indices(P)
    w = integral_coeffs.reshape(CELLS, len(ii)).astype(np.float64)
    S = np.zeros((CELLS, P, P), np.float64)
    # quadratic form: off-diag split in half, diag gets full coeff
    np.add.at(S, (slice(None), ii, jj), 0.5 * w)
    np.add.at(S, (slice(None), jj, ii), 0.5 * w)
    lam, V = np.linalg.eigh(S)  # V columns are eigenvectors

    lam_p = np.zeros((CELLS_PAD, P))
    lam_p[:CELLS] = lam
    V_p = np.zeros((CELLS_PAD, P, P))
    V_p[:CELLS] = V

    # block-diagonal stationary: wb[g, 8t+i, 8t+k] = V[16g+t, i, k]
    Vg = V_p.reshape(NGROUPS, GROUP_CELLS, P, P)
    wb = np.zeros((NGROUPS, GROUP_CELLS, P, GROUP_CELLS, P), np.float32)
    t = np.arange(GROUP_CELLS)
    wb[:, t, :, t, :] = Vg.transpose(1, 0, 2, 3)
    wblk = (
        wb.reshape(NGROUPS, PARTS, PARTS)
        .transpose(1, 0, 2)
        .reshape(PARTS, NGROUPS * PARTS)
        .astype(np.float16)
    )
    lamt = np.ascontiguousarray(
        lam_p.reshape(NGROUPS, PARTS).T.astype(np.float16)
    )
    return np.ascontiguousarray(wblk), lamt


def _host_prep_param(param_tensor):
    """param [B,G,G,P] f32 -> list of per-core [128, NGROUPS*NB] fp16 arrays."""
    flat = param_tensor.reshape(B, CELLS * P)
    out = []
    for c in range(NCORES):
        shard = flat[c * NB : (c + 1) * NB]
        pad = np.zeros((NB, CELLS_PAD * P), np.float32)
        pad[:, : CELLS * P] = shard
        # (b, g, p) -> (p, g, b)
        pt = (
            pad.reshape(NB, NGROUPS, PARTS)
            .transpose(2, 1, 0)
            .reshape(PARTS, NGROUPS * NB)
            .astype(np.float16)
        )
        out.append(np.ascontiguousarray(pt))
    return out


def _run(param_tensor, integral_coeffs, trace=False, **run_kwargs):
    from concourse.bass_utils import run_bass_kernel_spmd

    nc = _get_nc()
    wblk, lamt = _host_prep_weights(np.asarray(integral_coeffs, np.float32))
    pts = _host_prep_param(np.asarray(param_tensor, np.float32))
    in_maps = [{"pt": pts[c], "wblk": wblk, "lamt": lamt} for c in range(NCORES)]
    res = run_bass_kernel_spmd(
        nc, in_maps, core_ids=list(range(NCORES)), trace=trace, **run_kwargs
    )
    out = np.concatenate(
        [res.results[c]["out"].reshape(NB) for c in range(NCORES)]
    ).astype(np.float32)
    return out, res


def kernel(param_tensor, integral_coeffs):
    out, _ = _run(param_tensor, integral_coeffs)
    return out
